# revision 1
# baseline (speedup 1.0000x reference)
"""BiBloSAN Trainium2 kernel.

Shapes: B=4, N=16 blocks, R=64 tokens/block, D=256.
Sharding: one (batch, direction) pair per core -> 8 cores, no collectives.
The bw direction runs the SAME SPMD program on a host-reversed token
sequence (flat reverse maps the j<i mask onto the j>i program exactly).

Layout on device: feature dim d on partitions (2 tiles of 128), tokens on
the free axis. All matmuls are out[m,n] = sum_k lhsT[k,m] rhs[k,n] with
lhsT = weight matrices stored (in,out) as provided.
"""

import numpy as np
from contextlib import ExitStack

import concourse.bass as bass
import concourse.mybir as mybir
import concourse.tile as tile
from concourse import bacc, bass_utils

F32 = mybir.dt.float32
F16 = mybir.dt.float16
F32R = mybir.dt.float32r
AF = mybir.ActivationFunctionType

B, NB, R, D = 4, 16, 64, 256
T = NB * R          # 1024 tokens
DT = D // 128       # 2 partition tiles of feature dim
C = 5.0
NCORES = 8
ICH = 16            # i-chunk size in the intra-block mSA
NCH = R // ICH      # 4 chunks
GB = 4              # blocks per instruction group in the mSA loop


def _ap(t, offset, dims):
    """Raw AP on sbuf tile t: dims = [[step, count], ...] free dims."""
    base = t[:]
    return bass.AP(tensor=base.tensor, offset=base.offset + offset,
                   ap=[list(base.ap[0])] + [list(d) for d in dims])


def build_nc():
    nc = bacc.Bacc("TRN2", target_bir_lowering=False, debug=False,
                   num_devices=NCORES)

    # ---- DRAM I/O ----
    xT_d = nc.dram_tensor("xT", [D, T], F32R, kind="ExternalInput").ap()
    w_d = {}
    for nm in ("fcW", "mW1", "mW2", "s2tW1", "s2tW", "gW1", "gW2"):
        dt_ = F32R if nm in ("fcW", "mW1", "mW2", "s2tW1", "s2tW") else F32
        w_d[nm] = nc.dram_tensor(nm, [D, D], dt_, kind="ExternalInput").ap()
    for nm in ("fW1", "fW2"):
        w_d[nm] = nc.dram_tensor(nm, [3 * D, D], F32, kind="ExternalInput").ap()
    b_d = {}
    for nm in ("fcb", "mb", "s2tb1", "s2tb", "gb", "fb1", "fb2"):
        b_d[nm] = nc.dram_tensor(nm, [D], F32, kind="ExternalInput").ap()
    diag16_d = nc.dram_tensor("diag16", [128, ICH * ICH], F16,
                              kind="ExternalInput").ap()
    sela_d = nc.dram_tensor("sela", [128, 2560], F32R, kind="ExternalInput").ap()
    selb_d = nc.dram_tensor("selb", [128, 2560], F32R, kind="ExternalInput").ap()
    blkm_d = nc.dram_tensor("blkmask", [128, NB * NB], F32,
                            kind="ExternalInput").ap()
    eps64_d = nc.dram_tensor("eps64", [128, R], F32, kind="ExternalInput").ap()
    eps16_d = nc.dram_tensor("eps16", [128, NB], F32, kind="ExternalInput").ap()
    out_d = nc.dram_tensor("outT", [D, 32], F32, kind="ExternalOutput").ap()

    with tile.TileContext(nc) as tc, ExitStack() as ctx:
        const = ctx.enter_context(tc.tile_pool(name="const", bufs=1))
        big = ctx.enter_context(tc.tile_pool(name="big", bufs=1))
        work = ctx.enter_context(tc.tile_pool(name="work", bufs=2))
        psum = ctx.enter_context(
            tc.tile_pool(name="psum", bufs=1, space="PSUM"))
        xijps_pool = ctx.enter_context(
            tc.tile_pool(name="xijps", bufs=1, space="PSUM"))
        ups_pool = ctx.enter_context(
            tc.tile_pool(name="ups", bufs=2, space="PSUM"))
        xijsb_pool = ctx.enter_context(tc.tile_pool(name="xijsb", bufs=6))
        small = ctx.enter_context(tc.tile_pool(name="small", bufs=4))

        # ---- load weights / constants (one DMA per tensor) ----
        # order matters: FC/mSA weights + xT first, fusion weights last
        wsb = {}
        def load_w(nm, nkt=2):
            t = const.tile([128, nkt * D], w_d[nm].dtype, tag=nm)
            nc.sync.dma_start(
                out=t[:].rearrange("p (kt e) -> p kt e", kt=nkt),
                in_=w_d[nm].rearrange("(kt p) e -> p kt e", p=128))
            wsb[nm] = t
        for nm in ("fcW", "mW1", "mW2"):
            load_w(nm)
        bsb = {}
        for nm in ("fcb", "mb", "s2tb1", "s2tb", "gb", "fb1", "fb2"):
            t = const.tile([128, DT], F32, tag=nm)
            nc.sync.dma_start(out=t[:],
                              in_=b_d[nm].rearrange("(dt p) -> p dt", p=128))
            bsb[nm] = t
        mbC = const.tile([128, DT], F32)
        nc.scalar.mul(mbC[:], bsb["mb"][:], 1.0 / C)

        diag16 = const.tile([128, ICH * ICH], F16)
        nc.sync.dma_start(out=diag16[:], in_=diag16_d[:, :])
        sela = const.tile([128, 2560], F32R)
        nc.sync.dma_start(out=sela[:], in_=sela_d[:, :])
        selb = const.tile([128, 2560], F32R)
        nc.sync.dma_start(out=selb[:], in_=selb_d[:, :])
        blkm = const.tile([128, NB * NB], F32)
        nc.sync.dma_start(out=blkm[:], in_=blkm_d[:, :])
        eps64 = const.tile([128, R], F32)
        nc.sync.dma_start(out=eps64[:], in_=eps64_d[:, :])
        eps16 = const.tile([128, NB], F32)
        nc.sync.dma_start(out=eps16[:], in_=eps16_d[:, :])

        xT = big.tile([128, DT, T], F32R, tag="xT")
        for hf in range(2):
            for dt in range(DT):
                nc.sync.dma_start(
                    out=xT[:, dt, hf * 512:(hf + 1) * 512],
                    in_=xT_d[dt * 128:(dt + 1) * 128, hf * 512:(hf + 1) * 512])
        for nm in ("s2tW1", "s2tW", "gW1", "gW2"):
            load_w(nm)
        for nm in ("fW1", "fW2"):
            load_w(nm, nkt=6)

        # ---- helper: out[dt][:, :] = act(sum_k W[k-tiles].T @ rhs_fn(kt) + bias) ----
        def mm_all(dst, wname, rhs_fn, nkt, bias=None, func=AF.Copy,
                   nch_size=512, ncols=T, scale=1.0, ncs0=0):
            # dst: [128, DT, ncols] sbuf tile; lhsT = wsb[wname]
            w = wsb[wname]
            for ncs in range(ncs0, ncs0 + ncols, nch_size):
                for mt in range(DT):
                    ncols_i = min(nch_size, ncs0 + ncols - ncs)
                    pt = psum.tile([128, 512], F32, tag="mmps")
                    for kt in range(nkt):
                        nc.tensor.matmul(
                            pt[:, :ncols_i],
                            w[:, kt * D + mt * 128: kt * D + (mt + 1) * 128],
                            rhs_fn(kt, ncs, ncols_i),
                            start=(kt == 0), stop=(kt == nkt - 1))
                    if bias is not None:
                        nc.scalar.activation(dst[:, mt, ncs:ncs + ncols_i],
                                             pt[:, :ncols_i], func,
                                             bias=bias[:, mt:mt + 1])
                    else:
                        nc.vector.tensor_copy(dst[:, mt, ncs:ncs + ncols_i],
                                              pt[:, :ncols_i])

        # ---- FC: in_pT = relu(fcW.T @ xT + fcb) ----
        inp = big.tile([128, DT, T], F32R)
        mm_all(inp, "fcW", lambda kt, ncs, ncol: xT[:, kt, ncs:ncs + ncol],
               DT, bias=bsb["fcb"], func=AF.Relu)

        inpH = big.tile([128, DT, T], F16)
        for dt in range(DT):
            nc.vector.tensor_copy(inpH[:, dt, :], inp[:, dt, :].bitcast(F32))


        # ---- intra-block mSA ----
        # pair-vector layout per (blk, dt): triangle chunks c=0..3, chunk c is
        # an [ICH, jw] block of (i, j) pairs; offsets below.
        POFF = (0, 1024, 1792, 2304)
        PJW = (64, 48, 32, 16)
        NPAIR = 2560
        ND = big.tile([128, DT, 2, T], F32, tag="xT")   # [...,0,:]=num, [...,1,:]=den
        hT = big.tile([128, DT, T], F32R)
        fT = big.tile([128, DT, T], F32R, tag="xiT")
        eT = big.tile([128, DT, T], F32, tag="xjT")
        SUMS = small.tile([128, DT, NB], F32)
        NUMV = small.tile([128, DT, NB], F32)

        NGRP = NB // GB
        NPR = GB // 2
        for g in range(NGRP):
            # xi/xj for 2 blocks at a time: [128 tokens, 256 e] each
            xi2, xj2 = [], []
            for p2 in range(NPR):
                tok0 = (g * GB + p2 * 2) * R
                for dst_l, wname in ((xi2, "mW1"), (xj2, "mW2")):
                    xps = xijps_pool.tile([128, D], F32, tag="xijps")
                    for kt in range(DT):
                        nc.tensor.matmul(
                            xps[:], inp[:, kt, tok0:tok0 + 128],
                            wsb[wname][:, kt * D:(kt + 1) * D],
                            start=(kt == 0), stop=(kt == DT - 1))
                    xsb = xijsb_pool.tile([128, D], F32R, tag="xijsb")
                    nc.scalar.copy(xsb[:], xps[:])
                    dst_l.append(xsb)
            for dt in range(DT):
                # [bg, 0, :] = w, [bg, 1, :] = w*x
                w16 = work.tile([128, GB, 2, NPAIR], F16, tag="w16")
                for bg in range(GB):
                    p0 = bg % 2 * 64
                    xi_l = xi2[bg // 2][p0:p0 + 64, dt * 128:(dt + 1) * 128]
                    xj_l = xj2[bg // 2][p0:p0 + 64, dt * 128:(dt + 1) * 128]
                    # selI lives at rows p0 in sela (even) / selb (odd);
                    # selJ at rows p0 in selb (even) / sela (odd)
                    si = sela if bg % 2 == 0 else selb
                    sj = selb if bg % 2 == 0 else sela
                    for half in range(2):
                        up = ups_pool.tile([128, 1280], F32, tag="ups")
                        base = half * 1280
                        for n0, nw in ((0, 512), (512, 512), (1024, 256)):
                            nc.tensor.matmul(
                                up[:, n0:n0 + nw], xi_l,
                                si[p0:p0 + 64, base + n0:base + n0 + nw],
                                start=True, stop=False)
                            nc.tensor.matmul(
                                up[:, n0:n0 + nw], xj_l,
                                sj[p0:p0 + 64, base + n0:base + n0 + nw],
                                start=False, stop=True)
                        nc.scalar.activation(
                            w16[:, bg, 0, base:base + 1280], up[:], AF.Tanh,
                            bias=mbC[:, dt:dt + 1], scale=1.0 / C)
                # exp over the w-halves (split per block-pair for pipelining)
                for bp in range(GB // 2):
                    wall = _ap(w16, bp * 2 * 2 * NPAIR,
                               [[2 * NPAIR, 2], [1, NPAIR]])
                    nc.scalar.activation(wall, wall, AF.Exp, scale=C)
                for c in range(NCH):
                    jw = PJW[c]
                    goff = POFF[c]
                    # diagonal mask on first ICH j-cols of the chunk
                    dmw = _ap(w16, goff, [[2 * NPAIR, GB], [jw, ICH], [1, ICH]])
                    dm = _ap(diag16, 0, [[0, GB], [ICH, ICH], [1, ICH]])
                    nc.vector.tensor_mul(dmw, dmw, dm)
                for c in range(NCH):
                    jw = PJW[c]
                    goff = POFF[c]
                    tok = g * GB * R + c * ICH
                    # wx = w * x  (fp16 2x mult)
                    wv = _ap(w16, goff, [[2 * NPAIR, GB], [jw, ICH], [1, jw]])
                    wxv = _ap(w16, NPAIR + goff,
                              [[2 * NPAIR, GB], [jw, ICH], [1, jw]])
                    xv_ap = _ap(inpH, dt * T + tok, [[R, GB], [0, ICH], [1, jw]])
                    nc.vector.tensor_mul(wxv, wv, xv_ap)
                    # merged fold chain over (w, wx) halves
                    nd_ap = bass.AP(
                        tensor=ND[:].tensor, offset=ND[:].offset + dt * 2 * T + tok,
                        ap=[list(ND[:].ap[0]), [R, GB], [T, 2], [1, ICH]])
                    wcur = jw
                    while wcur > 1 and wcur % 2 == 0:
                        h = wcur // 2
                        a0 = _ap(w16, goff,
                                 [[2 * NPAIR, GB], [NPAIR, 2], [jw, ICH], [1, h]])
                        a1 = _ap(w16, goff + h,
                                 [[2 * NPAIR, GB], [NPAIR, 2], [jw, ICH], [1, h]])
                        if h == 1:
                            nc.vector.tensor_add(nd_ap, a0, a1)
                        else:
                            nc.vector.tensor_add(a0, a0, a1)
                        wcur = h
                    if wcur > 1:    # odd remainder (e.g. 3 for jw=48)
                        nc.vector.tensor_reduce(
                            nd_ap,
                            _ap(w16, goff,
                                [[2 * NPAIR, GB], [NPAIR, 2], [jw, ICH],
                                 [1, wcur]]),
                            mybir.AxisListType.X, mybir.AluOpType.add)

            # ---- per-group epilogue: h, s2t softmax and block summary ----
            GC = GB * R                              # 256 token cols
            g0 = g * GC
            for dt in range(DT):
                epsf = _ap(eps64, 0, [[0, GB], [1, R]])
                nc.vector.tensor_add(ND[:, dt, 0, g0:g0 + GC],
                                     ND[:, dt, 0, g0:g0 + GC], epsf)
                nc.vector.reciprocal(ND[:, dt, 0, g0:g0 + GC],
                                     ND[:, dt, 0, g0:g0 + GC])
                nc.vector.tensor_mul(hT[:, dt, g0:g0 + GC],
                                     ND[:, dt, 1, g0:g0 + GC],
                                     ND[:, dt, 0, g0:g0 + GC])
            # s2t for this group's 4 blocks
            for mt in range(DT):
                ptf = psum.tile([128, GC], F32, tag="mmps")
                for kt in range(DT):
                    nc.tensor.matmul(
                        ptf[:],
                        wsb["s2tW1"][:, kt * D + mt * 128: kt * D + (mt + 1) * 128],
                        hT[:, kt, g0:g0 + GC], start=(kt == 0),
                        stop=(kt == DT - 1))
                nc.scalar.activation(fT[:, mt, g0:g0 + GC], ptf[:], AF.Relu,
                                     bias=bsb["s2tb1"][:, mt:mt + 1])
            for mt in range(DT):
                pte = psum.tile([128, GC], F32, tag="mmps")
                for kt in range(DT):
                    nc.tensor.matmul(
                        pte[:],
                        wsb["s2tW"][:, kt * D + mt * 128: kt * D + (mt + 1) * 128],
                        fT[:, kt, g0:g0 + GC], start=(kt == 0),
                        stop=(kt == DT - 1))
                nc.scalar.activation(eT[:, mt, g0:g0 + GC], pte[:], AF.Exp,
                                     bias=bsb["s2tb"][:, mt:mt + 1])
            for dt in range(DT):
                nc.vector.tensor_reduce(
                    SUMS[:, dt, g * GB:(g + 1) * GB],
                    eT[:, dt, g0:g0 + GC].rearrange("p (n r) -> p n r", r=R),
                    mybir.AxisListType.X, mybir.AluOpType.add)
                wh = work.tile([128, GC], F32, tag="wh")
                nc.vector.tensor_mul(wh[:], eT[:, dt, g0:g0 + GC],
                                     hT[:, dt, g0:g0 + GC].bitcast(F32))
                nc.vector.tensor_reduce(
                    NUMV[:, dt, g * GB:(g + 1) * GB],
                    wh[:].rearrange("p (n r) -> p n r", r=R),
                    mybir.AxisListType.X, mybir.AluOpType.add)
        vT = small.tile([128, DT, NB], F32)
        for dt in range(DT):
            nc.vector.reciprocal(SUMS[:, dt, :], SUMS[:, dt, :])
            nc.vector.tensor_mul(vT[:, dt, :], NUMV[:, dt, :], SUMS[:, dt, :])

        # ---- block-level mSA over v (rows computed for all 16) ----
        viT = small.tile([128, DT, NB], F32)
        vjT = small.tile([128, DT, NB], F32)
        for dst, wname in ((viT, "mW1"), (vjT, "mW2")):
            w = wsb[wname]
            for mt in range(DT):
                pt = psum.tile([128, NB], F32, tag="mmps")
                for kt in range(DT):
                    nc.tensor.matmul(
                        pt[:],
                        w[:, kt * D + mt * 128: kt * D + (mt + 1) * 128]
                        .bitcast(F32),
                        vT[:, kt, :], start=(kt == 0), stop=(kt == DT - 1))
                nc.vector.tensor_copy(dst[:, mt, :], pt[:])
        oT = small.tile([128, DT, NB], F32)
        ub = work.tile([128, DT, NB, NB], F32, tag="ublk")
        # u[dt,i,j] = vi[dt,i] + vj[dt,j]
        vi2 = _ap(viT, 0, [[NB, DT], [1, NB], [0, NB]])
        vj2 = _ap(vjT, 0, [[NB, DT], [0, NB], [1, NB]])
        nc.vector.tensor_add(ub[:], vi2, vj2)
        for dt in range(DT):
            nc.scalar.activation(ub[:, dt], ub[:, dt], AF.Tanh,
                                 bias=mbC[:, dt:dt + 1], scale=1.0 / C)
        nc.scalar.activation(ub[:], ub[:], AF.Exp, scale=C)
        bm = _ap(blkm, 0, [[0, DT], [NB, NB], [1, NB]])
        nc.vector.tensor_mul(ub[:], ub[:], bm)
        deno = small.tile([128, DT, NB], F32, tag="deno")
        nc.vector.tensor_reduce(deno[:], ub[:], mybir.AxisListType.X,
                                mybir.AluOpType.add)
        nc.vector.tensor_add(deno[:], deno[:],
                             _ap(eps16, 0, [[0, DT], [1, NB]]))
        wv = work.tile([128, DT, NB, NB], F32, tag="wv")
        nc.vector.tensor_mul(wv[:], ub[:],
                             _ap(vT, 0, [[NB, DT], [0, NB], [1, NB]]))
        numo = small.tile([128, DT, NB], F32, tag="numo")
        nc.vector.tensor_reduce(numo[:], wv[:], mybir.AxisListType.X,
                                mybir.AluOpType.add)
        nc.vector.reciprocal(deno[:], deno[:])
        nc.vector.tensor_mul(oT[:], numo[:], deno[:])

        # ---- gating at rows 0 and 15 ----
        o01 = small.tile([128, DT, 2], F32)
        v01 = small.tile([128, DT, 2], F32)
        for dt in range(DT):
            nc.vector.tensor_copy(o01[:, dt, :],
                                  _ap(oT, dt * NB, [[NB - 1, 2]]))
            nc.vector.tensor_copy(v01[:, dt, :],
                                  _ap(vT, dt * NB, [[NB - 1, 2]]))
        G01 = small.tile([128, DT, 2], F32)
        for mt in range(DT):
            pt = psum.tile([128, 2], F32, tag="mmps")
            for kt in range(DT):
                nc.tensor.matmul(
                    pt[:], wsb["gW1"][:, kt * D + mt * 128: kt * D + (mt + 1) * 128],
                    o01[:, kt, :], start=(kt == 0), stop=False)
            for kt in range(DT):
                nc.tensor.matmul(
                    pt[:], wsb["gW2"][:, kt * D + mt * 128: kt * D + (mt + 1) * 128],
                    v01[:, kt, :], start=False, stop=(kt == DT - 1))
            nc.scalar.activation(G01[:, mt, :], pt[:], AF.Sigmoid,
                                 bias=bsb["gb"][:, mt:mt + 1])
        e01 = small.tile([128, DT, 2], F32)
        for dt in range(DT):
            tmp = small.tile([128, 2], F32, tag="etmp")
            nc.vector.tensor_sub(tmp[:], o01[:, dt, :], v01[:, dt, :])
            nc.vector.tensor_mul(tmp[:], tmp[:], G01[:, dt, :])
            nc.vector.tensor_add(e01[:, dt, :], v01[:, dt, :], tmp[:])

        # ---- fusion for both candidate slices ----
        # slice A: cols 0:16 with E=e01[...,0]; slice B: cols 1008:1024, E=e01[...,1]
        EA = small.tile([128, DT, 2, 16], F32)   # [dt, slice, 16]
        for dt in range(DT):
            for s in range(2):
                nc.vector.tensor_copy(EA[:, dt, s, :],
                                      _ap(e01, dt * 2 + s, [[0, 16]]))
        outT = small.tile([128, DT, 32], F32)
        scol = (0, T - 16)
        for wname, bname, func, dstname in (("fW1", "fb1", AF.Relu, "fus"),
                                            ("fW2", "fb2", AF.Sigmoid, "gf")):
            dst = small.tile([128, DT, 32], F32, tag=dstname)
            if dstname == "fus":
                fus = dst
            else:
                gf = dst
            for mt in range(DT):
                for s in range(2):
                    c0 = scol[s]
                    pt = psum.tile([128, 16], F32, tag="mmps")
                    for kt in range(6):
                        if kt < 2:
                            rhs = inp[:, kt, c0:c0 + 16].bitcast(F32)
                        elif kt < 4:
                            rhs = hT[:, kt - 2, c0:c0 + 16].bitcast(F32)
                        else:
                            rhs = EA[:, kt - 4, s, :]
                        nc.tensor.matmul(
                            pt[:],
                            wsb[wname][:, kt * D + mt * 128: kt * D + (mt + 1) * 128],
                            rhs, start=(kt == 0), stop=(kt == 5))
                    nc.scalar.activation(dst[:, mt, s * 16:(s + 1) * 16], pt[:],
                                         func, bias=bsb[bname][:, mt:mt + 1])
        for mt in range(DT):
            for s in range(2):
                xf = inp[:, mt, scol[s]:scol[s] + 16].bitcast(F32)
                of = outT[:, mt, s * 16:(s + 1) * 16]
                nc.vector.tensor_sub(of, fus[:, mt, s * 16:(s + 1) * 16], xf)
                nc.vector.tensor_mul(of, of, gf[:, mt, s * 16:(s + 1) * 16])
                nc.vector.tensor_add(of, of, xf)
        for mt in range(DT):
            nc.sync.dma_start(out=out_d[mt * 128:(mt + 1) * 128, :],
                              in_=outT[:, mt, :])
    nc.compile()
    return nc


_NC = None


def _get_nc():
    global _NC
    if _NC is None:
        _NC = build_nc()
    return _NC


def _consts():
    il = np.arange(ICH)
    diag = (il[None, :] > il[:, None]).astype(np.float16).reshape(-1)
    diagmask = np.broadcast_to(diag, (128, ICH * ICH)).copy()
    bi = np.arange(NB)
    blk = (bi[None, :] > bi[:, None]).astype(np.float32).reshape(-1)
    blkmask = np.broadcast_to(blk, (128, NB * NB)).copy()
    e64 = np.zeros(R, np.float32); e64[R - 1] = 1.0
    eps64 = np.broadcast_to(e64, (128, R)).copy()
    e16 = np.zeros(NB, np.float32); e16[NB - 1] = 1.0
    eps16 = np.broadcast_to(e16, (128, NB)).copy()
    selI = np.zeros((64, 2560), np.float32)
    selJ = np.zeros((64, 2560), np.float32)
    col = 0
    for c in range(NCH):
        for il in range(ICH):
            for jl in range(R - ICH * c):
                selI[ICH * c + il, col] = 1.0
                selJ[ICH * c + jl, col] = 1.0
                col += 1
    assert col == 2560
    sela = np.concatenate([selI, selJ], 0)
    selb = np.concatenate([selJ, selI], 0)
    return diagmask, blkmask, eps64, eps16, sela, selb


def prep_in_maps(inputs):
    x = np.asarray(inputs["x"], np.float32)
    diagmask, blkmask, eps64, eps16, sela, selb = _consts()
    wnames = ("fcW", "mW1", "mW2", "s2tW1", "s2tW", "gW1", "gW2", "fW1", "fW2")
    bnames = ("fcb", "mb", "s2tb1", "s2tb", "gb", "fb1", "fb2")

    in_maps = []
    for core in range(NCORES):
        b = core % B
        sfx = "_fw" if core < B else "_bw"
        xf = x[b].reshape(T, D)
        if core >= B:
            xf = xf[::-1]
        m = {"xT": np.ascontiguousarray(xf.T),
             "diag16": diagmask, "blkmask": blkmask,
             "eps64": eps64, "eps16": eps16, "sela": sela, "selb": selb}
        for nm in wnames:
            m[nm] = np.ascontiguousarray(inputs[nm + sfx], np.float32)
        for nm in bnames:
            m[nm] = np.ascontiguousarray(inputs[nm + sfx], np.float32)
        in_maps.append(m)
    return in_maps


def assemble(outs):
    u_fw = np.stack([outs[b]["outT"][:, 0:16].T for b in range(B)])
    u_bw = np.stack([outs[B + b]["outT"][:, 16:32].T[::-1] for b in range(B)])
    return np.concatenate([u_fw, u_bw], axis=-1).astype(np.float32)


def kernel(**inputs):
    in_maps = prep_in_maps(inputs)
    res = bass_utils.run_bass_kernel_spmd(_get_nc(), in_maps,
                                          core_ids=list(range(NCORES)))
    return assemble(res.results)

